# revision 13
# baseline (speedup 1.0000x reference)
"""Trainium2 Bass kernel for DicGaussianRBF.

out = concat([ones(N,1), data, exp(-5 * ||data - centers||^2)], axis=-1)
with data [65536, 256] f32, centers [2048, 256] f32 -> out [65536, 2305] f32.

Data-parallel over N across 8 NeuronCores; centers replicated. Per core
(8192 rows, 64 row-blocks of 128). The kernel is HBM-bandwidth bound
(~86 MB of DMA per core vs ~358 GB/s per-core HBM), so every engine is
kept far below the DMA roofline even when the PE HAM clock-gate throttles
the tensor engine to 1.2 GHz:

  setup: centersT cast to fp8e4 via PE transpose + DVE copy; c2 row via
  ones-matmul of bf16 squares, broadcast to 128 partitions via a rank-1
  matmul and scaled to c2m5 = -5*||c||^2 [128, 2048] f32.

  steady state (software-skewed by PRE row-blocks):
    - input staged 8 row-blocks (1 MB) per SWDGE DMA into `din` tiles.
    - per row-block: DVE computes bias = -5*||x||^2 in one
      scalar_tensor_tensor; PE transposes the data tile; DVE casts it to
      fp8e4 with a -2 scale; 4 fp8 DoubleRow matmuls (full 256-dim
      contraction each) write psum = -2 x.c; DVE folds in the center
      norms: arg = -5*psum + c2m5; GpSimd fills [ones | data] into the
      full-row output tile; ScalarE evaluates exp(arg + bias) into its
      rbf columns; one dense HWDGE DMA writes the full 128x2305 row block
      (sequential 1.18 MB HBM span, 9220-byte descriptors).
"""

import sys

for _p in ("/opt/trn_rl_repo",):
    if _p not in sys.path:
        sys.path.insert(0, _p)

import numpy as np

import concourse.bass as bass
import concourse.tile as tile
from concourse import bacc, mybir
from concourse import bass_utils
from concourse.masks import make_identity

N, D, K = 65536, 256, 2048
NCORES = 8
N_LOC = N // NCORES          # 8192 rows per core
OUT_W = 1 + D + K            # 2305
RB = N_LOC // 128            # 64 row blocks per core
SB = 8                       # row blocks per input staging DMA
PRE = 4                      # transpose pipeline lookahead (row blocks)
S = 5.0

FP32 = mybir.dt.float32
BF16 = mybir.dt.bfloat16
FP8 = mybir.dt.float8e4
Act = mybir.ActivationFunctionType
MULT = mybir.AluOpType.mult
ADD = mybir.AluOpType.add
DR = mybir.MatmulPerfMode.DoubleRow

_cached_nc = None


def _build():
    nc = bacc.Bacc(
        "TRN2",
        target_bir_lowering=False,
        debug=False,
        enable_asserts=False,
        num_devices=NCORES,
    )
    data_ap = nc.dram_tensor("data", [N_LOC, D], FP32, kind="ExternalInput").ap()
    cent_ap = nc.dram_tensor("centers", [K, D], FP32, kind="ExternalInput").ap()
    out_ap = nc.dram_tensor("out", [N_LOC, OUT_W], FP32, kind="ExternalOutput").ap()

    with tile.TileContext(nc) as tc:
        with (
            tc.tile_pool(name="const", bufs=1) as const,
            tc.tile_pool(name="cload", bufs=1) as cload,
            tc.tile_pool(name="dinp", bufs=5) as dinp,
            tc.tile_pool(name="rbfp", bufs=6) as rbfp,
            tc.tile_pool(name="dtp", bufs=6) as dtp,
            tc.tile_pool(name="scrp", bufs=3) as scrp,
            tc.tile_pool(name="biasp", bufs=8) as biasp,
            tc.tile_pool(name="argp", bufs=6) as argp,
            tc.tile_pool(name="pstr", bufs=2, space="PSUM") as pstr,
            tc.tile_pool(name="psmm", bufs=3, space="PSUM") as psmm,
        ):
            ident = const.tile([128, 128], FP32)
            make_identity(nc, ident)
            ones_r1 = const.tile([1, 128], BF16)
            nc.vector.memset(ones_r1[:], 1.0)
            ones_col = const.tile([128, 1], BF16)
            nc.vector.memset(ones_col[:], 1.0)

            # centersT: [128, 2K] fp8; [:, 0:K] = dims 0:128, [:, K:2K] = 128:256
            cTi = const.tile([128, 2 * K], FP8)
            sq = [
                const.tile([128, K], BF16, name=f"sq{b}", tag=f"sq{b}")
                for b in range(2)
            ]
            c2row = const.tile([1, K], BF16)
            c2m5 = const.tile([128, K], FP32)

            din_tiles = {}

            def load_super_block(sb, dma_engine=None):
                din = dinp.tile([128, SB * D], FP32, tag="din", name="din")
                din_tiles[sb] = din
                din3 = din[:].rearrange("p (r d) -> p r d", d=D)
                src = data_ap[sb * SB * 128:(sb + 1) * SB * 128, :].rearrange(
                    "(r p) d -> p r d", p=128
                )
                (dma_engine or nc.gpsimd).dma_start(din3[:, :, :], src)

            # setup loads ride the idle ACT HWDGE queue: faster first byte and
            # no Q7 descriptor-generation serialization; centers split in 4
            # chunks so the transpose pipeline starts on the first 512 KB
            call = cload.tile([128, (K // 128) * D], FP32)
            call3 = call[:].rearrange("p (t d) -> p t d", d=D)
            load_super_block(0)
            for c in range(4):
                csrc = cent_ap[c * 512:(c + 1) * 512, :].rearrange(
                    "(t p) d -> p t d", p=128
                )
                nc.scalar.dma_start(call3[:, c * 4:(c + 1) * 4, :], csrc)
                if c == 1:
                    load_super_block(1)

            for i in range(K // 128):
                ct = call[:, i * D:(i + 1) * D]
                pt = pstr.tile([128, 256], FP32, tag="pt")
                nc.tensor.transpose(pt[:, 0:128], ct[:, 0:128], ident[:])
                nc.tensor.transpose(pt[:, 128:256], ct[:, 128:256], ident[:])
                nc.vector.tensor_copy(cTi[:, i * 128:(i + 1) * 128], pt[:, 0:128])
                nc.vector.tensor_copy(cTi[:, K + i * 128:K + (i + 1) * 128], pt[:, 128:256])
                if i % 4 == 3:
                    # c2 for this 512-wide chunk of centers: column sums of the
                    # squared fp8 centersT, then broadcast to 128 partitions
                    kb = i // 4
                    ks = slice(kb * 512, (kb + 1) * 512)
                    for b in range(2):
                        cslice = cTi[:, b * K + kb * 512:b * K + (kb + 1) * 512]
                        nc.vector.tensor_mul(sq[b][:, ks], cslice, cslice)
                    pc = pstr.tile([1, 512], FP32, tag="pt", name="pc")
                    nc.tensor.matmul(pc[:], ones_col[:], sq[0][:, ks], start=True, stop=False)
                    nc.tensor.matmul(pc[:], ones_col[:], sq[1][:, ks], start=False, stop=True)
                    nc.vector.tensor_copy(c2row[:, ks], pc[:])
                    bc = pstr.tile([128, 512], FP32, tag="pt", name="bc")
                    nc.tensor.matmul(bc[:], ones_r1[:], c2row[:, ks], start=True, stop=True)
                    nc.vector.tensor_scalar_mul(c2m5[:, ks], bc[:], -S)

            stage = {}
            for step in range(RB + PRE):
                # ---- front of the pipe: stage input, bias, transpose, cast
                rb = step
                if rb < RB:
                    if rb % SB == 0 and rb // SB + 2 < RB // SB:
                        load_super_block(rb // SB + 2)
                    din = din_tiles[rb // SB]
                    b = rb % SB
                    dcol = din[:, b * D:(b + 1) * D]

                    scratch = scrp.tile([128, D], BF16, tag="scr")
                    bias = biasp.tile([128, 1], FP32, tag="bias")
                    nc.vector.scalar_tensor_tensor(
                        scratch[:], dcol, -S, dcol, MULT, MULT, accum_out=bias[:]
                    )

                    pt = pstr.tile([128, 256], FP32, tag="pt")
                    nc.tensor.transpose(pt[:, 0:128], dcol[:, 0:128], ident[:])
                    nc.tensor.transpose(pt[:, 128:256], dcol[:, 128:256], ident[:])
                    dT = dtp.tile([128, 256], FP8, tag="dT")
                    nc.vector.tensor_scalar_mul(dT[:], pt[:], -2.0)
                    stage[rb] = (dT, bias, dcol)

                # ---- back of the pipe: matmuls, c2 fold, exp, output DMA
                rbm = step - PRE
                if rbm >= 0:
                    dT, bias, dcolm = stage.pop(rbm)
                    rs = slice(rbm * 128, (rbm + 1) * 128)
                    # full-row output tile [1 | data | rbf]: the write to HBM
                    # is one dense, perfectly sequential 1.18 MB span of
                    # 9220-byte descriptors. GpSimd fills the ones+data
                    # columns; ScalarE writes the rbf block.
                    ot = rbfp.tile([128, OUT_W], FP32, tag="ot")
                    nc.gpsimd.memset(ot[:, 0:1], 1.0)
                    nc.scalar.copy(ot[:, 1:257], dcolm)
                    dTv = dT[:].rearrange("p (two m) -> p two m", two=2)
                    cTv = cTi[:].rearrange("p (two k) -> p two k", two=2)
                    for half in range(2):
                        ps = psmm.tile([128, 1024], FP32, tag="mm")
                        for q in range(2):
                            ks = slice(half * 1024 + q * 512, half * 1024 + (q + 1) * 512)
                            nc.tensor.matmul(
                                ps[:, q * 512:(q + 1) * 512],
                                dTv,
                                cTv[:, :, ks],
                                start=True,
                                stop=True,
                                perf_mode=DR,
                            )
                        hs = slice(half * 1024, (half + 1) * 1024)
                        arg = argp.tile([128, 1024], FP32, tag="arg")
                        nc.vector.scalar_tensor_tensor(
                            arg[:], ps[:], -S, c2m5[:, hs], MULT, ADD
                        )
                        nc.scalar.activation(
                            ot[:, 257 + half * 1024:257 + (half + 1) * 1024],
                            arg[:],
                            Act.Exp,
                            bias=bias[:],
                            scale=1.0,
                        )
                        if rbm >= RB - 2:
                            # tail: fire each half as soon as its exp lands so
                            # the final DMA isn't serialized behind both halves
                            cs = slice(0, 1281) if half == 0 else slice(1281, OUT_W)
                            nc.sync.dma_start(out_ap[rs, cs], ot[:, cs])
                    if rbm < RB - 2:
                        nc.sync.dma_start(out_ap[rs, :], ot[:])

    nc.compile()
    return nc


def _get_nc():
    global _cached_nc
    if _cached_nc is None:
        _cached_nc = _build()
    return _cached_nc


def kernel(data, centers):
    data = np.ascontiguousarray(np.asarray(data, dtype=np.float32))
    centers = np.ascontiguousarray(np.asarray(centers, dtype=np.float32))
    assert data.shape == (N, D) and centers.shape == (K, D)

    nc = _get_nc()
    in_maps = [
        {"data": data[i * N_LOC:(i + 1) * N_LOC], "centers": centers}
        for i in range(NCORES)
    ]
    res = bass_utils.run_bass_kernel_spmd(nc, in_maps, core_ids=list(range(NCORES)))
    return np.concatenate([res.results[i]["out"] for i in range(NCORES)], axis=0)
